# revision 1
# baseline (speedup 1.0000x reference)
"""Trainium2 Bass kernel for nn_DFTQNN: 8 sequential gates psi <- expm(-i*theta_g*G_g) @ psi,
output |psi|^2.

Algorithm: instead of materializing expm (matrix-matrix, ~20 GFLOP/gate), apply each gate's
exponential directly to the statevector with a Chebyshev expansion:

    exp(-i t G) v = sum_k c_k T_k(G/lam) v,   c_k = mu_k (-i sgn)^k J_k(|t| lam)

where lam >= ||G||_2 (host-side power iteration), J_k = Bessel. The T_k(G/lam) v iterates
satisfy w_{k+1} = (2/lam) G w_k - w_{k-1} -- a chain of ~6-10 1024x1024 matvecs per gate
(~60 total), ~1000x fewer flops than the reference expm path.

Device mapping (per NeuronCore, fully replicated across the 8 cores -- the chain is
serial and cross-core collectives cost ~5us/call, far more than they could save):
  - statevector w kept in "chunk layout" [128 part, 16 free]: partition r, col 2q+s
    holds component s (re/im) of element m = 128q + r.
  - per term: y[m] = sum_n G[n, m] w[n] (G symmetric) via 32 accumulating fp32r matmuls:
    8 contraction chunks x 4 column-tiled groups (tile_position=(0,32j), N=256 streams
    run concurrently on separate XBUSes; fp32r streams 1 row/cycle at N>=256).
  - y lands free-major in PSUM [2-of-32 part rows, 256]; ScalarE copies it to SBUF with
    the 2/lam Chebyshev scale folded into the activation scale; 8 PE transposes
    [2,128] -> [128,2] return it to chunk layout; VectorE applies the three-term
    recurrence and the c_k accumulation (pure-real/pure-imag alternation = strided
    column APs, no cross-partition work).
"""

import os
import numpy as np

DIM = 1024
P = 128
NCH = DIM // P        # 8 contraction chunks
NGRP = 4              # column-tile groups
GW = DIM // NGRP      # 256 stream columns per group
NGATE = 8
CHEB_TOL = 1e-4
KMAX = 48


# ---------------------------------------------------------------- host math
def _bessel_j(kmax, z, npts=2048):
    t = np.linspace(0.0, np.pi, npts + 1)
    k = np.arange(kmax + 1)[:, None]
    f = np.cos(k * t[None, :] - z * np.sin(t)[None, :])
    return np.trapezoid(f, t, axis=1) / np.pi


def _lam_max(G, iters=60, seed=0):
    rng = np.random.default_rng(seed)
    v = rng.standard_normal(G.shape[0])
    v /= np.linalg.norm(v)
    lam = 1.0
    for _ in range(iters):
        w = G @ v
        lam = np.linalg.norm(w)
        v = w / lam
    return float(lam)


def _cheb_coeffs(theta, lam, tol=CHEB_TOL, kmax=KMAX):
    z = theta * lam
    J = _bessel_j(kmax, abs(z))
    sgn = 1.0 if z >= 0 else -1.0
    c = np.zeros(kmax + 1, dtype=np.complex128)
    c[0] = J[0]
    for k in range(1, kmax + 1):
        c[k] = 2.0 * ((-1j * sgn) ** k) * J[k]
    mags = np.abs(c)
    K = 0
    acc = 0.0
    for k in range(kmax, 0, -1):
        acc += mags[k]
        if acc > tol:
            K = min(k + 1, kmax)
            break
    return c, K


# ---------------------------------------------------------------- device program
def build_program(lams, coeffs, Ks):
    import concourse.bass as bass
    import concourse.mybir as mybir
    import concourse.tile as tile
    from concourse import bacc
    from contextlib import ExitStack

    f32 = mybir.dt.float32
    f32r = mybir.dt.float32r
    Copy = mybir.ActivationFunctionType.Copy
    Alu = mybir.AluOpType

    nc = bacc.Bacc("TRN2", target_bir_lowering=False, debug=False, num_devices=8)

    feat_d = nc.dram_tensor("feature", [DIM], f32, kind="ExternalInput")
    gens_d = nc.dram_tensor("gens", [NGATE, DIM, DIM], f32, kind="ExternalInput")
    out_d = nc.dram_tensor("out", [DIM], f32, kind="ExternalOutput")

    with tile.TileContext(nc) as tc, ExitStack() as ctx:
        const = ctx.enter_context(tc.tile_pool(name="const", bufs=1))
        gpool = ctx.enter_context(tc.tile_pool(name="gpool", bufs=3))
        spool = ctx.enter_context(tc.tile_pool(name="spool", bufs=2))
        wpool = ctx.enter_context(tc.tile_pool(name="wpool", bufs=3))
        apool = ctx.enter_context(tc.tile_pool(name="apool", bufs=2))
        pspool = ctx.enter_context(tc.tile_pool(name="ps", bufs=1, space="PSUM"))
        psmall = ctx.enter_context(tc.tile_pool(name="pssm", bufs=1, space="PSUM"))

        # ---- constants
        id_t = const.tile([2, 2], f32)
        nc.gpsimd.memset(id_t[:], 0.0)
        nc.gpsimd.affine_select(
            out=id_t[:],
            in_=id_t[:],
            compare_op=Alu.not_equal,
            fill=1.0,
            base=0,
            pattern=[[-1, 2]],
            channel_multiplier=1,
        )
        ones_col = const.tile([P, 1], f32)
        nc.vector.memset(ones_col[:], 1.0)
        ones_row = const.tile([1, P], f32)
        nc.vector.memset(ones_row[:], 1.0)

        # ---- feature embedding + 1/||f||^2
        f_emb = apool.tile([P, 2 * NCH], f32, tag="acc")
        nc.vector.memset(f_emb[:], 0.0)
        f_even = f_emb[:].rearrange("p (q s) -> p q s", s=2)[:, :, 0]
        # feature[m], m = 128 q + r  ->  partition r, col q
        nc.sync.dma_start(out=f_even, in_=feat_d.ap().rearrange("(q r) -> r q", r=P))

        sq = const.tile([P, NCH], f32)
        nc.vector.tensor_tensor(out=sq[:], in0=f_even, in1=f_even, op=Alu.mult)
        rsum = const.tile([P, 1], f32)
        nc.vector.tensor_reduce(
            out=rsum[:], in_=sq[:], axis=mybir.AxisListType.X, op=Alu.add
        )
        n2_ps = psmall.tile([1, 1], f32, tag="n2")
        nc.tensor.matmul(out=n2_ps[:], lhsT=ones_col[:], rhs=rsum[:], start=True, stop=True)
        inv_sb = const.tile([1, 1], f32)
        nc.vector.reciprocal(out=inv_sb[:], in_=n2_ps[:])
        invb_ps = psmall.tile([P, 1], f32, tag="invb")
        nc.tensor.matmul(out=invb_ps[:], lhsT=ones_row[:], rhs=inv_sb[:], start=True, stop=True)
        invb = const.tile([P, 1], f32)
        nc.vector.tensor_copy(invb[:], invb_ps[:])

        # ---- persistent PSUM tiles (fully written each term; no memset needed)
        Y = [pspool.tile([2, 512], f32, tag=f"Y{i}", name=f"Y{i}") for i in range(2)]
        Tt = pspool.tile([P, 2 * NCH], f32, tag="T", name="Tps")

        # ---- gate chain
        w_cur = f_emb  # w_0 of gate 0 (acc-pool tile)
        term_idx = 0
        for g in range(NGATE):
            lam = lams[g]
            c = coeffs[g]
            K = Ks[g]

            # raw G chunks: gt[c][k, m] = G[128 c + k, m]; DMA fp32 staging then
            # cast to the fp32r encoding on the Scalar engine
            gts = []
            for ch in range(NCH):
                gs = gpool.tile([P, DIM], f32, tag=f"Gs{ch}", name=f"Gs{g}_{ch}")
                nc.sync.dma_start(out=gs[:], in_=gens_d.ap()[g, 128 * ch : 128 * (ch + 1), :])
                gt = gpool.tile([P, DIM], f32r, tag=f"G{ch}", name=f"G{g}_{ch}")
                nc.scalar.activation(out=gt[:], in_=gs[:], func=Copy)
                gts.append(gt)

            acc = apool.tile([P, 2 * NCH], f32, tag="acc", name=f"acc{g}")
            # acc = c0 * w0
            nc.vector.tensor_scalar_mul(acc[:], w_cur[:], float(c[0].real))

            w_prev = None
            for k in range(1, K + 1):
                Sa = spool.tile([2, 512], f32, tag="Sa", name=f"Sa{term_idx}")
                Sb = spool.tile([2, 512], f32, tag="Sb", name=f"Sb{term_idx}")
                w_r = wpool.tile([P, 2 * NCH], f32r, tag="wr", name=f"wr{term_idx}")
                nc.vector.tensor_copy(w_r[:, 0:8], w_cur[:, 0:8])
                nc.vector.tensor_copy(w_r[:, 8:16], w_cur[:, 8:16])
                term_idx += 1

                # y = G^T w  (16 accumulating fp32r matmuls, N=512, two PSUM banks)
                # half-major order: Y[0]'s copy/transposes overlap Y[1]'s matmuls
                scale = (1.0 / lam) if k == 1 else (2.0 / lam)
                w_next = wpool.tile([P, 2 * NCH], f32, tag="w", name=f"w{g}_{k}")
                for half in range(2):
                    for ch in range(NCH):
                        nc.tensor.matmul(
                            out=Y[half][:],
                            lhsT=w_r[:, 2 * ch : 2 * ch + 2],
                            rhs=gts[ch][:, 512 * half : 512 * (half + 1)].bitcast(f32r),
                            start=(ch == 0),
                            stop=(ch == NCH - 1),
                            tile_position=(0, 0),
                        )
                    if half == 0:
                        # S = (2/lam) y  (k==1: 1/lam); DVE while PE streams half 1
                        nc.vector.tensor_scalar_mul(Sa[:], Y[0][:], float(scale))
                    else:
                        # ACT, split in two so transposes q=4,5 start sooner
                        nc.scalar.activation(out=Sb[:, 0:256], in_=Y[1][:, 0:256], func=Copy, scale=float(scale))
                        nc.scalar.activation(out=Sb[:, 256:512], in_=Y[1][:, 256:512], func=Copy, scale=float(scale))

                # 8 transposes [2,128] -> [128,2] back to chunk layout, then the
                # recurrence per half so next term's first matmuls start early
                for q in range(2 * NCH // 2):
                    Sx = Sa if q < 4 else Sb
                    h = q % 4
                    nc.tensor.matmul(
                        Tt[:, 2 * q : 2 * q + 2],
                        Sx[:, P * h : P * (h + 1)],
                        id_t[:],
                        is_transpose=True,
                        start=(q == 0),
                        stop=(q == NCH - 1),
                        tile_position=(0, 0),
                    )
                    if q == 3 or q == NCH - 1:
                        cols = slice(0, 8) if q == 3 else slice(8, 16)
                        if k == 1:
                            nc.vector.tensor_copy(w_next[:, cols], Tt[:, cols])
                        else:
                            nc.vector.tensor_tensor(
                                out=w_next[:, cols], in0=Tt[:, cols],
                                in1=w_prev[:, cols], op=Alu.subtract,
                            )

                # acc += c_k * sigma_k(w)  (fused multiply-add on DVE)
                ck = c[k]
                if k % 2 == 0:
                    nc.vector.scalar_tensor_tensor(
                        out=acc[:], in0=w_next[:], scalar=float(ck.real),
                        in1=acc[:], op0=Alu.mult, op1=Alu.add,
                    )
                else:
                    wv = w_next[:].rearrange("p (q s) -> p q s", s=2)
                    av = acc[:].rearrange("p (q s) -> p q s", s=2)
                    b = float(ck.imag)
                    nc.vector.scalar_tensor_tensor(
                        out=av[:, :, 0], in0=wv[:, :, 1], scalar=-b,
                        in1=av[:, :, 0], op0=Alu.mult, op1=Alu.add,
                    )
                    nc.vector.scalar_tensor_tensor(
                        out=av[:, :, 1], in0=wv[:, :, 0], scalar=b,
                        in1=av[:, :, 1], op0=Alu.mult, op1=Alu.add,
                    )

                w_prev, w_cur = w_cur, w_next

            w_cur = acc  # unnormalized psi after gate g

        # ---- output: |psi|^2 / ||f||^2
        sq2 = const.tile([P, 2 * NCH], f32)
        nc.vector.tensor_tensor(out=sq2[:], in0=w_cur[:], in1=w_cur[:], op=Alu.mult)
        sv = sq2[:].rearrange("p (q s) -> p q s", s=2)
        prob = const.tile([P, NCH], f32)
        nc.vector.tensor_tensor(out=prob[:], in0=sv[:, :, 0], in1=sv[:, :, 1], op=Alu.add)
        nc.vector.tensor_scalar_mul(prob[:], prob[:], invb[:])
        nc.sync.dma_start(out=out_d.ap().rearrange("(q r) -> r q", r=P), in_=prob[:])

    nc.compile()
    return nc


# ---------------------------------------------------------------- entry point
_CACHE = {}


def _prep(theta, gens):
    lams = [_lam_max(gens[g].astype(np.float64)) * 1.03 for g in range(NGATE)]
    coeffs, Ks = [], []
    for g in range(NGATE):
        c, K = _cheb_coeffs(float(theta[g, 0]), lams[g])
        coeffs.append(c)
        Ks.append(max(K, 1))
    return lams, coeffs, Ks


def kernel(feature, theta, gens):
    from concourse.bass_utils import run_bass_kernel_spmd

    feature = np.ascontiguousarray(feature, dtype=np.float32)
    theta = np.ascontiguousarray(theta, dtype=np.float32)
    gens = np.ascontiguousarray(gens, dtype=np.float32)

    lams, coeffs, Ks = _prep(theta, gens)
    key = (theta.tobytes(), tuple(np.round(lams, 9)), tuple(Ks))
    if key not in _CACHE:
        _CACHE[key] = build_program(lams, coeffs, Ks)
    nc = _CACHE[key]

    in_map = {"feature": feature, "gens": gens}
    res = run_bass_kernel_spmd(
        nc,
        [dict(in_map) for _ in range(8)],
        core_ids=list(range(8)),
        trace=False,
    )
    return np.asarray(res.results[0]["out"], dtype=np.float32)


if __name__ == "__main__":
    d = np.load("/root/problem/ref_cache.npz")
    out = kernel(d["feature"], d["theta"], d["gens"])
    exp = d["expected"]
    rel = np.linalg.norm(out - exp) / np.linalg.norm(exp)
    print("l2 rel err:", rel)
    print("max abs err:", np.abs(out - exp).max())



# revision 3
# speedup vs baseline: 1.1416x; 1.1416x over previous
"""Trainium2 Bass kernel for nn_DFTQNN: 8 sequential gates psi <- expm(-i*theta_g*G_g) @ psi,
output |psi|^2.

Chebyshev expansion applied directly to the statevector (see v1 docstring): per gate,
K matvecs w_{k+1} = (2/lam) G w_k - w_{k-1}; acc = sum_k c_k w_k with c_k Bessel coeffs.

v2 changes vs v1 (baseline 348us):
  - G pre-scaled by 2/lam_g and converted to fp16 ON HOST -> no on-device fp32->fp32r
    ACT casts (was ~66us of Scalar time), DMA bytes halved, and fp16 streams 1 col/cycle
    at any N. All 64 [128,1024] G chunk tiles SBUF-resident (128KB/partition).
  - w state kept in fp16 chunk layout; recurrence output written fp16 directly by DVE
    (no separate fp32r copy ops).
  - PE instruction stream reordered to be gap-free so the PE p-state ramps to 2.4GHz and
    stays (trace showed most matmuls at 1.2GHz): next term's first contraction chunks are
    emitted between this term's half-0 and half-1 transposes, hiding the PSUM->SBUF copy
    and recurrence latency under real matmul work.
  - Chebyshev tolerance 1e-4 -> 2e-3 (47 -> 38 terms); measured rel err stays ~1e-3,
    budget is 2e-2.

Fully replicated across the 8 cores (the chain is serial; cross-core collectives cost
more than they save).
"""

import numpy as np

DIM = 1024
P = 128
NCH = DIM // P        # 8 contraction chunks
NGATE = 8
CHEB_TOL = 2e-3
KMAX = 48


# ---------------------------------------------------------------- host math
def _bessel_j(kmax, z, npts=2048):
    t = np.linspace(0.0, np.pi, npts + 1)
    k = np.arange(kmax + 1)[:, None]
    f = np.cos(k * t[None, :] - z * np.sin(t)[None, :])
    return np.trapezoid(f, t, axis=1) / np.pi


def _lam_max(G, iters=60, seed=0):
    rng = np.random.default_rng(seed)
    v = rng.standard_normal(G.shape[0])
    v /= np.linalg.norm(v)
    lam = 1.0
    for _ in range(iters):
        w = G @ v
        lam = np.linalg.norm(w)
        v = w / lam
    return float(lam)


def _cheb_coeffs(theta, lam, tol=CHEB_TOL, kmax=KMAX):
    z = theta * lam
    J = _bessel_j(kmax, abs(z))
    sgn = 1.0 if z >= 0 else -1.0
    c = np.zeros(kmax + 1, dtype=np.complex128)
    c[0] = J[0]
    for k in range(1, kmax + 1):
        c[k] = 2.0 * ((-1j * sgn) ** k) * J[k]
    mags = np.abs(c)
    K = 0
    acc = 0.0
    for k in range(kmax, 0, -1):
        acc += mags[k]
        if acc > tol:
            K = min(k + 1, kmax)
            break
    return c, K


# ---------------------------------------------------------------- device program
def build_program(lams, coeffs, Ks):
    import concourse.bass as bass
    import concourse.mybir as mybir
    import concourse.tile as tile
    from concourse import bacc
    from contextlib import ExitStack

    f32 = mybir.dt.float32
    f16 = mybir.dt.float16
    Alu = mybir.AluOpType

    nc = bacc.Bacc("TRN2", target_bir_lowering=False, debug=False, num_devices=8)

    feat_d = nc.dram_tensor("feature", [DIM], f32, kind="ExternalInput")
    gens_d = nc.dram_tensor("gens16", [NGATE, DIM, DIM], f16, kind="ExternalInput")
    out_d = nc.dram_tensor("out", [DIM], f32, kind="ExternalOutput")

    with tile.TileContext(nc) as tc, ExitStack() as ctx:
        const = ctx.enter_context(tc.tile_pool(name="const", bufs=1))
        gpool = ctx.enter_context(tc.tile_pool(name="gpool", bufs=1))
        spool = ctx.enter_context(tc.tile_pool(name="spool", bufs=2))
        wpool = ctx.enter_context(tc.tile_pool(name="wpool", bufs=3))
        apool = ctx.enter_context(tc.tile_pool(name="apool", bufs=2))
        pspool = ctx.enter_context(tc.tile_pool(name="ps", bufs=1, space="PSUM"))
        psmall = ctx.enter_context(tc.tile_pool(name="pssm", bufs=1, space="PSUM"))

        # ---- all G chunk tiles resident in SBUF, DMA'd up front in use order
        gts = []  # gts[g][ch] : [128, 1024] f16
        for g in range(NGATE):
            row = []
            for ch in range(NCH):
                gt = gpool.tile([P, DIM], f16, tag=f"G{g}_{ch}", name=f"G{g}_{ch}")
                nc.sync.dma_start(
                    out=gt[:], in_=gens_d.ap()[g, 128 * ch : 128 * (ch + 1), :]
                )
                row.append(gt)
            gts.append(row)

        # ---- constants
        id_t = const.tile([2, 2], f32)
        nc.gpsimd.memset(id_t[:], 0.0)
        nc.gpsimd.affine_select(
            out=id_t[:],
            in_=id_t[:],
            compare_op=Alu.not_equal,
            fill=1.0,
            base=0,
            pattern=[[-1, 2]],
            channel_multiplier=1,
        )
        ones_col = const.tile([P, 1], f32)
        nc.vector.memset(ones_col[:], 1.0)
        ones_row = const.tile([1, P], f32)
        nc.vector.memset(ones_row[:], 1.0)

        # ---- feature embedding + 1/||f||^2
        f_emb = const.tile([P, 2 * NCH], f32)
        nc.vector.memset(f_emb[:], 0.0)
        f_even = f_emb[:].rearrange("p (q s) -> p q s", s=2)[:, :, 0]
        # feature[m], m = 128 q + r  ->  partition r, col q
        nc.sync.dma_start(out=f_even, in_=feat_d.ap().rearrange("(q r) -> r q", r=P))

        sq = const.tile([P, NCH], f32)
        nc.vector.tensor_tensor(out=sq[:], in0=f_even, in1=f_even, op=Alu.mult)
        rsum = const.tile([P, 1], f32)
        nc.vector.tensor_reduce(
            out=rsum[:], in_=sq[:], axis=mybir.AxisListType.X, op=Alu.add
        )
        n2_ps = psmall.tile([1, 1], f32, tag="n2")
        nc.tensor.matmul(out=n2_ps[:], lhsT=ones_col[:], rhs=rsum[:], start=True, stop=True)
        inv_sb = const.tile([1, 1], f32)
        nc.vector.reciprocal(out=inv_sb[:], in_=n2_ps[:])
        invb_ps = psmall.tile([P, 1], f32, tag="invb")
        nc.tensor.matmul(out=invb_ps[:], lhsT=ones_row[:], rhs=inv_sb[:], start=True, stop=True)
        invb = const.tile([P, 1], f32)
        nc.vector.tensor_copy(invb[:], invb_ps[:])

        # ---- persistent PSUM tiles
        Y = [pspool.tile([2, 512], f32, tag=f"Y{i}", name=f"Y{i}") for i in range(2)]
        Tt = pspool.tile([P, 2 * NCH], f32, tag="T", name="Tps")

        # ================================================================
        # Gate chain.  Per term the PE stream is (steady state):
        #   [mm h0 c0..7 -> Y0][mm h1 c0..7 -> Y1]
        #   [tr h0 q0..3]                      (Sa copied during mm h1)
        #   [mm' h0 c0..3 -> Y0]               (w cols 0:8 from rec h0)
        #   [tr h1 q4..7]                      (Sb copy hidden under mm' h0 c0..3)
        #   [mm' h1 c0..3 -> Y1]
        #   [mm' h0 c4..7 -> Y0]               (w cols 8:16 from rec h1)
        #   [mm' h1 c4..7 -> Y1] ...
        # DVE: S copies (low half), recurrences.  ACT: S copies (high half), acc FMAs.
        # ================================================================

        # emit the matvec matmuls for (gate g, term tag t) into Y, split in two
        # chunk groups so the caller can interleave; returns list of closures
        def emit_mm(g, w16, half, c_lo, c_hi):
            first = c_lo == 0
            last = c_hi == NCH
            for c in range(c_lo, c_hi):
                nc.tensor.matmul(
                    out=Y[half][:],
                    lhsT=w16[:, 2 * c : 2 * c + 2],
                    rhs=gts[g][c][:, 512 * half : 512 * (half + 1)],
                    start=(c == c_lo) and first,
                    stop=(c == c_hi - 1) and last,
                    skip_group_check=True,
                )

        def emit_tr(Sx, q_lo, q_hi, t):
            # transposes [2,128] -> [128,2], each its own write (no accumulation)
            for q in range(q_lo, q_hi):
                h = q % 4
                nc.tensor.matmul(
                    Tt[:, 2 * q : 2 * q + 2],
                    Sx[:, P * h : P * (h + 1)],
                    id_t[:],
                    is_transpose=True,
                    start=True,
                    stop=True,
                    skip_group_check=True,
                )

        def emit_copy(Sx, half, t):
            # PSUM -> SBUF staging, split DVE (low 256) / ACT (high 256)
            nc.vector.tensor_copy(Sx[:, 0:256], Y[half][:, 0:256])
            nc.scalar.activation(
                out=Sx[:, 256:512], in_=Y[half][:, 256:512],
                func=mybir.ActivationFunctionType.Copy,
            )

        def emit_rec(w_new16, w_prev16, k, cols):
            # w_new = Tt - w_prev   (k==1: w_new = 0.5*Tt)
            if k == 1:
                nc.vector.tensor_scalar_mul(w_new16[:, cols], Tt[:, cols], 0.5)
            else:
                nc.vector.tensor_tensor(
                    out=w_new16[:, cols], in0=Tt[:, cols],
                    in1=w_prev16[:, cols], op=Alu.subtract,
                )

        def emit_acc(acc, w16, ck, k):
            # acc += c_k * w  (DVE, after the recurrences in program order)
            if k % 2 == 0:
                nc.vector.scalar_tensor_tensor(
                    out=acc[:], in0=w16[:], scalar=float(ck.real),
                    in1=acc[:], op0=Alu.mult, op1=Alu.add,
                )
            else:
                wv = w16[:].rearrange("p (q s) -> p q s", s=2)
                av = acc[:].rearrange("p (q s) -> p q s", s=2)
                b = float(ck.imag)
                nc.vector.scalar_tensor_tensor(
                    out=av[:, :, 0], in0=wv[:, :, 1], scalar=-b,
                    in1=av[:, :, 0], op0=Alu.mult, op1=Alu.add,
                )
                nc.vector.scalar_tensor_tensor(
                    out=av[:, :, 1], in0=wv[:, :, 0], scalar=b,
                    in1=av[:, :, 1], op0=Alu.mult, op1=Alu.add,
                )

        term_idx = 0
        w_cur_f32 = f_emb  # gate 0 input state (fp32 [128,16])
        for g in range(NGATE):
            c = coeffs[g]
            K = Ks[g]

            # w0 in fp16 + acc init = c0 * w0
            w0 = wpool.tile([P, 2 * NCH], f16, tag="w", name=f"w0_{g}")
            nc.vector.tensor_copy(w0[:], w_cur_f32[:])
            acc = apool.tile([P, 2 * NCH], f32, tag="acc", name=f"acc{g}")
            nc.vector.tensor_scalar_mul(acc[:], w_cur_f32[:], float(c[0].real))

            w_prev, w_cur = None, w0
            # pipelined term loop: term k's matmuls are interleaved with term
            # k-1's half-1 transpose/recurrence so the PE never waits:
            #   [mm h0 c0-3][tr h1(k-1)][mm h1 c0-3][mm h0 c4-7][mm h1 c4-7]
            # (mm *h0* c4-7 needs w cols 8:16 <- rec h1(k-1), ready by then;
            #  mm h1 c0-3 needs only cols 0:8 and hides that latency)
            pend = None  # (Sb, k, w_new, w_prev) of previous term
            for k in range(1, K + 1):
                t = term_idx
                term_idx += 1
                Sa = spool.tile([2, 512], f32, tag="Sa", name=f"Sa{t}")
                Sb = spool.tile([2, 512], f32, tag="Sb", name=f"Sb{t}")
                w_new = wpool.tile([P, 2 * NCH], f16, tag="w", name=f"w{g}_{k}")

                if pend is None:
                    # first term of the gate: plain order
                    emit_mm(g, w_cur, 0, 0, 4)
                    emit_mm(g, w_cur, 1, 0, 4)
                    emit_mm(g, w_cur, 0, 4, NCH)
                    emit_mm(g, w_cur, 1, 4, NCH)
                else:
                    pSb, pk, pw_new, pw_prev = pend
                    emit_mm(g, w_cur, 0, 0, 4)
                    emit_tr(pSb, 4, 8, t - 1)
                    emit_rec(pw_new, pw_prev, pk, slice(8, 16))
                    emit_acc(acc, pw_new, c[pk], pk)
                    emit_mm(g, w_cur, 1, 0, 4)
                    emit_mm(g, w_cur, 0, 4, NCH)
                    emit_mm(g, w_cur, 1, 4, NCH)
                emit_copy(Sa, 0, t)
                emit_copy(Sb, 1, t)
                emit_tr(Sa, 0, 4, t)
                emit_rec(w_new, w_prev, k, slice(0, 8))
                pend = (Sb, k, w_new, w_prev)
                w_prev, w_cur = w_cur, w_new

            # drain the last term's half-1 pipeline
            pSb, pk, pw_new, pw_prev = pend
            emit_tr(pSb, 4, 8, term_idx - 1)
            emit_rec(pw_new, pw_prev, pk, slice(8, 16))
            emit_acc(acc, pw_new, c[pk], pk)

            # gate output (fp32) becomes next gate's input state
            w_cur_f32 = acc

        # ---- output: |psi|^2 / ||f||^2
        sq2 = const.tile([P, 2 * NCH], f32)
        nc.vector.tensor_tensor(out=sq2[:], in0=w_cur_f32[:], in1=w_cur_f32[:], op=Alu.mult)
        sv = sq2[:].rearrange("p (q s) -> p q s", s=2)
        prob = const.tile([P, NCH], f32)
        nc.vector.tensor_tensor(out=prob[:], in0=sv[:, :, 0], in1=sv[:, :, 1], op=Alu.add)
        nc.vector.tensor_scalar_mul(prob[:], prob[:], invb[:])
        nc.sync.dma_start(out=out_d.ap().rearrange("(q r) -> r q", r=P), in_=prob[:])

    nc.compile()
    return nc


# ---------------------------------------------------------------- entry point
_CACHE = {}


def _prep(theta, gens):
    lams = [_lam_max(gens[g].astype(np.float64)) * 1.03 for g in range(NGATE)]
    coeffs, Ks = [], []
    for g in range(NGATE):
        c, K = _cheb_coeffs(float(theta[g, 0]), lams[g])
        coeffs.append(c)
        Ks.append(max(K, 1))
    return lams, coeffs, Ks


def _gens16(gens, lams):
    # fold the Chebyshev 2/lam scale into the fp16 G
    g16 = np.empty(gens.shape, dtype=np.float16)
    for g in range(NGATE):
        g16[g] = (gens[g] * np.float32(2.0 / lams[g])).astype(np.float16)
    return g16


def kernel(feature, theta, gens):
    from concourse.bass_utils import run_bass_kernel_spmd

    feature = np.ascontiguousarray(feature, dtype=np.float32)
    theta = np.ascontiguousarray(theta, dtype=np.float32)
    gens = np.ascontiguousarray(gens, dtype=np.float32)

    lams, coeffs, Ks = _prep(theta, gens)
    key = (theta.tobytes(), tuple(np.round(lams, 9)), tuple(Ks))
    if key not in _CACHE:
        _CACHE[key] = build_program(lams, coeffs, Ks)
    nc = _CACHE[key]

    in_map = {"feature": feature, "gens16": _gens16(gens, lams)}
    res = run_bass_kernel_spmd(
        nc,
        [dict(in_map) for _ in range(8)],
        core_ids=list(range(8)),
        trace=False,
    )
    return np.asarray(res.results[0]["out"], dtype=np.float32)


if __name__ == "__main__":
    d = np.load("/root/problem/ref_cache.npz")
    out = kernel(d["feature"], d["theta"], d["gens"])
    exp = d["expected"]
    rel = np.linalg.norm(out - exp) / np.linalg.norm(exp)
    print("l2 rel err:", rel)
    print("max abs err:", np.abs(out - exp).max())
